# revision 1
# baseline (speedup 1.0000x reference)
import numpy as np
import jax
import jax.numpy as jnp

# nn_GaussianRayTracer: B=1, H=W=128 (R=16384 rays), N=1024 gaussians.
# Sharding: data-parallel over rays — the H*W ray axis is split across the
# 8 NeuronCores (pmap); gaussian attributes are replicated. Each core produces
# the per-(ray,gaussian) depth/alpha tensors (the memory-dominant [R,N] part);
# the per-ray sort + compositing (small, control-heavy, and trn2 has no sort
# HLO) runs on host in numpy.

B, H, W, N = 1, 128, 128, 1024
R = H * W
M = 8
RL = R // M
T_MIN = 1e-3
ALPHA_MIN = 1e-2


def _pair_fn(rdc, F, v, Q6, oo, opa):
    # rdc: [3,RL,1] ray dir components; F: [6,RL,1] quadratic ray features
    # v: [3,1,N]; Q6: [6,1,N]; oo: [N]; opa: [N]
    # Only broadcasted elementwise ops — stays in exact fp32 on device.
    dot_od = rdc[0] * v[0] + rdc[1] * v[1] + rdc[2] * v[2]            # [RL,N]
    dd = (F[0] * Q6[0] + F[1] * Q6[1] + F[2] * Q6[2]
          + F[3] * Q6[3] + F[4] * Q6[4] + F[5] * Q6[5])               # [RL,N]
    t = -dot_od / dd
    dist2 = oo[None, :] - dot_od * dot_od / dd
    alpha = jnp.minimum(opa[None, :] * jnp.exp(-0.5 * dist2), 0.999)
    valid = (t > 0.0) & (alpha > ALPHA_MIN)
    alpha = jnp.where(valid, alpha, 0.0)
    tm = jnp.where(valid, t, jnp.inf)
    return tm, alpha


_pmapped = jax.pmap(_pair_fn, in_axes=(0, 0, None, None, None, None))


def kernel(rgs_xyz, rgs_rot, rgs_sca, rgs_opa, rgs_rgb, rgs_nrm, bg_raw, ray_org, ray_dir):
    f32 = np.float32
    xyz = np.asarray(rgs_xyz, f32)[0]
    rot = np.asarray(rgs_rot, f32)[0]
    sca = np.asarray(rgs_sca, f32)[0]
    opa = np.asarray(rgs_opa, f32)[0, :, 0]
    rgb = np.asarray(rgs_rgb, f32)[0]
    nrm = np.asarray(rgs_nrm, f32)[0]
    ro = np.asarray(ray_org, f32).reshape(3)
    rd = np.asarray(ray_dir, f32).reshape(R, 3)
    bg = np.broadcast_to(np.asarray(bg_raw, f32), (B, H, W, 3)).reshape(R, 3)

    # --- host: tiny per-gaussian precompute (O(N)) ---
    q = rot / np.sqrt(np.sum(rot * rot, axis=-1, keepdims=True) + 1e-12)
    w_, x_, y_, z_ = q[:, 0], q[:, 1], q[:, 2], q[:, 3]
    Rm = np.stack([
        1 - 2 * (y_ * y_ + z_ * z_), 2 * (x_ * y_ - w_ * z_), 2 * (x_ * z_ + w_ * y_),
        2 * (x_ * y_ + w_ * z_), 1 - 2 * (x_ * x_ + z_ * z_), 2 * (y_ * z_ - w_ * x_),
        2 * (x_ * z_ - w_ * y_), 2 * (y_ * z_ + w_ * x_), 1 - 2 * (x_ * x_ + y_ * y_)],
        axis=-1).reshape(N, 3, 3).astype(f32)
    Minv = (np.swapaxes(Rm, -1, -2) / sca[:, :, None]).astype(f32)     # [N,3,3]
    o_loc = np.einsum('nij,nj->ni', Minv, ro[None, :] - xyz).astype(f32)
    v = np.einsum('nij,ni->nj', Minv, o_loc).astype(f32)               # [N,3]
    Q = np.einsum('nki,nkj->nij', Minv, Minv).astype(f32)              # [N,3,3]
    oo = np.sum(o_loc * o_loc, axis=-1).astype(f32)                    # [N]
    Q6 = np.stack([Q[:, 0, 0], Q[:, 1, 1], Q[:, 2, 2],
                   2 * Q[:, 0, 1], 2 * Q[:, 0, 2], 2 * Q[:, 1, 2]], axis=0).astype(f32)  # [6,N]

    dx, dy, dz = rd[:, 0], rd[:, 1], rd[:, 2]
    F = np.stack([dx * dx, dy * dy, dz * dz, dx * dy, dx * dz, dy * dz], axis=0).astype(f32)  # [6,R]

    # --- device: [R,N] pair tensors, sharded over rays across 8 cores ---
    rdc_sh = rd.T.reshape(3, M, RL, 1).transpose(1, 0, 2, 3)           # [M,3,RL,1]
    F_sh = F.reshape(6, M, RL, 1).transpose(1, 0, 2, 3)                # [M,6,RL,1]
    tm_d, alpha_d = _pmapped(jnp.asarray(rdc_sh), jnp.asarray(F_sh),
                             jnp.asarray(v.T.reshape(3, 1, N)),
                             jnp.asarray(Q6.reshape(6, 1, N)),
                             jnp.asarray(oo), jnp.asarray(opa))
    tm = np.asarray(tm_d).reshape(R, N)
    alpha = np.asarray(alpha_d).reshape(R, N)

    # --- host: per-ray front-to-back compositing (order-dependent part) ---
    order = np.argsort(tm, axis=-1, kind='stable')
    alpha_s = np.take_along_axis(alpha, order, axis=-1)
    cp = np.cumprod(1.0 - alpha_s, axis=-1, dtype=f32)
    Tb = np.concatenate([np.ones((R, 1), f32), cp[:, :-1]], axis=-1)
    w_s = alpha_s * Tb * (Tb > T_MIN)
    w = np.empty_like(w_s)
    np.put_along_axis(w, order, w_s, axis=-1)                          # gaussian order

    nrm_unit = nrm / np.sqrt(np.sum(nrm * nrm, axis=-1, keepdims=True) + 1e-12)
    t0 = np.where(np.isfinite(tm), tm, 0.0).astype(f32)
    img = w @ rgb
    nrm_acc = w @ nrm_unit.astype(f32)
    dep = np.sum(w * t0, axis=-1, keepdims=True)
    alpha_acc = np.sum(w, axis=-1, keepdims=True)
    image = img + (1.0 - alpha_acc) * bg
    normal = nrm_acc / np.sqrt(np.sum(nrm_acc * nrm_acc, axis=-1, keepdims=True) + 1e-12)
    out = np.concatenate([image, alpha_acc, dep, normal], axis=-1).astype(f32)
    return out.reshape(B, H, W, 8)



# revision 28
# speedup vs baseline: 74.4967x; 74.4967x over previous
import numpy as np

# nn_GaussianRayTracer: B=1, H=W=128 (R=16384 rays), N=1024 gaussians.
# Data-parallel over rays: 2048 rays per core on 8 NeuronCores, gaussian
# attributes replicated. The whole pipeline (pair depth/alpha, ordered
# compositing, output reductions) runs in ONE Bass kernel launch.
#
# Sort-free compositing: the reference sorts per ray by t and takes a
# cumprod of (1-alpha). Since the outputs are only per-ray reductions over
# gaussians, the per-gaussian transmittance is computed directly as
#   T_i = exp(S_i),  S_i = sum_j [t_j < t_i] * log(1 - alpha_j)
# which needs no sort. On the vector engine one fused scalar_tensor_tensor
# instruction per gaussian column computes S_i for 128 rays at once:
#   out_j = (k_j > k_i) * L_j ; accum_out = sum_j out_j
# (k = -t with invalids pushed to -BIG, so k_j > k_i  <=>  t_j < t_i).

B, H, W, N = 1, 128, 128, 1024
R = H * W
M = 8            # cores
RL = R // M      # rays per core
P = 128          # rays per tile (partition dim)
NT = RL // P     # tiles per core
T_MIN = 1e-3
ALPHA_MIN = 1e-2
BIG = 1e30

_rt = None


def _build_bass(rl, nt, split_waits=True):
    import concourse.bass as bass
    import concourse.mybir as mybir
    from concourse.tile import TileContext

    dt = mybir.dt
    f32 = dt.float32
    alu = mybir.AluOpType
    act = mybir.ActivationFunctionType
    X = mybir.AxisListType.X

    nc = bass.Bass()
    rdf = nc.dram_tensor("rdf", [P, nt * 9], f32, kind="ExternalInput")
    rows = nc.dram_tensor("rows", [16, N], f32, kind="ExternalInput")
    y = nc.dram_tensor("y", [P, nt * 8], f32, kind="ExternalOutput")

    with TileContext(nc) as tc:
        with (
            tc.tile_pool(name="const", bufs=1) as cpool,
            tc.tile_pool(name="work", bufs=1) as pool,
        ):
            # broadcast rows across partitions (single DMA):
            # 0: c_row, 1-3: rgb, 4-6: nrm_unit, 7-9: v, 10-15: Q6
            bc_all = cpool.tile([P, 16, N], f32)
            nc.gpsimd.dma_start(out=bc_all, in_=rows[:, :].partition_broadcast(P))
            bc = [bc_all[:, r, :] for r in range(16)]
            junk = cpool.tile([P, N], f32)
            # all per-ray inputs in one DMA: [P, nt*9]
            rdf_all = cpool.tile([P, nt * 9], f32)
            nc.gpsimd.dma_start(out=rdf_all, in_=rdf[:, :])
            # all outputs accumulate here; single DMA at the end
            y_all = cpool.tile([P, nt * 8], f32)
            # sentinels: absorb the DMA-sem waits into copy instructions so
            # downstream TensorScalarPtr ops (1 sync-wait slot in ISA) only
            # see same-engine ordering.
            scr = cpool.tile([P, 16], f32)
            nc.vector.tensor_copy(scr, bc_all[:, :, 0])
            scr9 = cpool.tile([P, 9], f32)
            nc.vector.tensor_copy(scr9, rdf_all[:, 0:9])
            zcol = cpool.tile([P, 1], f32)
            nc.vector.memset(zcol, 0.0)

            for it in range(nt):
                def rdfc(c):
                    return rdf_all[:, it * 9 + c : it * 9 + c + 1]

                # dot_od[p,n] = sum_c rd[p,c] * v[c,n]
                d0 = pool.tile([P, N], f32, tag="d0")
                nc.vector.tensor_scalar(d0, bc[7], rdfc(0), None, alu.mult)
                d1 = pool.tile([P, N], f32, tag="d1")
                nc.vector.scalar_tensor_tensor(
                    d1, bc[8], rdfc(1), d0, alu.mult, alu.add
                )
                pdot = pool.tile([P, N], f32, tag="pdot")
                nc.vector.scalar_tensor_tensor(
                    pdot, bc[9], rdfc(2), d1, alu.mult, alu.add
                )
                # dd[p,n] = sum_c f6[p,c] * Q6[c,n]
                e = pool.tile([P, N], f32, tag="e0")
                nc.vector.tensor_scalar(e, bc[10], rdfc(3), None, alu.mult)
                for c in range(1, 6):
                    e2 = pool.tile([P, N], f32, tag=f"e{c}")
                    nc.vector.scalar_tensor_tensor(
                        e2, bc[10 + c], rdfc(3 + c), e, alu.mult, alu.add
                    )
                    e = e2
                pdd = e
                # u = dot/dd ;  t = -u
                rdd = pool.tile([P, N], f32, tag="rdd")
                nc.vector.reciprocal(rdd, pdd)
                u = pool.tile([P, N], f32, tag="u")
                nc.vector.tensor_tensor(u, pdot, rdd, alu.mult)
                # alpha = min(exp(0.5*dot*u + (ln opa - 0.5 oo)), 0.999)
                q1 = pool.tile([P, N], f32, tag="q1")
                nc.vector.scalar_tensor_tensor(q1, u, 0.5, pdot, alu.mult, alu.mult)
                arg = pool.tile([P, N], f32, tag="arg")
                nc.vector.tensor_tensor(arg, q1, bc[0], alu.add)
                argc = pool.tile([P, N], f32, tag="argc")
                nc.vector.tensor_scalar(argc, arg, 0.0, None, alu.min)
                araw = pool.tile([P, N], f32, tag="araw")
                nc.scalar.activation(araw, argc, act.Exp, bias=zcol)
                aclip = pool.tile([P, N], f32, tag="aclip")
                nc.vector.tensor_scalar(aclip, araw, 0.999, None, alu.min)
                va0 = pool.tile([P, N], f32, tag="va0")
                nc.vector.scalar_tensor_tensor(
                    va0, aclip, ALPHA_MIN, aclip, alu.is_gt, alu.mult
                )
                negm = pool.tile([P, N], f32, tag="negm")
                nc.vector.tensor_scalar(negm, u, 0.0, None, alu.is_lt)  # t>0
                va = pool.tile([P, N], f32, tag="va")
                nc.vector.tensor_tensor(va, va0, negm, alu.mult)
                # L = ln(1 - va)
                om = pool.tile([P, N], f32, tag="om")
                nc.vector.tensor_scalar(om, va, -1.0, 1.0, alu.mult, alu.add)
                lt = pool.tile([P, N], f32, tag="lt")
                nc.scalar.activation(lt, om, act.Ln, bias=zcol)
                # k = u + (valid-1)*BIG ; tmv = t*valid = -u*valid
                validb = pool.tile([P, N], f32, tag="validb")
                nc.vector.tensor_scalar(validb, va, 0.0, None, alu.is_gt)
                vm1 = pool.tile([P, N], f32, tag="vm1")
                nc.vector.tensor_scalar(vm1, validb, BIG, -BIG, alu.mult, alu.add)
                k = pool.tile([P, N], f32, tag="k")
                nc.vector.tensor_tensor(k, vm1, u, alu.add)
                tmv = pool.tile([P, N], f32, tag="tmv")
                nc.vector.scalar_tensor_tensor(tmv, validb, -1.0, u, alu.mult, alu.mult)
                # pairwise: S[:, i] = sum_j (k_j > k_i) * L_j
                s = pool.tile([P, N], f32, tag="s")
                # sentinel: absorb the ACT(lt)/WAR(s) waits into a copy so the
                # stt stream below needs no sync fields.
                nc.vector.tensor_copy(s[:, 0:1], lt[:, 0:1])
                for i in range(N):
                    nc.vector.scalar_tensor_tensor(
                        junk,
                        k,
                        k[:, i : i + 1],
                        lt,
                        alu.is_gt,
                        alu.mult,
                        accum_out=s[:, i : i + 1],
                    )
                # w = va * T * (T > T_MIN),  T = exp(S)
                tt_ = pool.tile([P, N], f32, tag="tt")
                nc.scalar.activation(tt_, s, act.Exp, bias=zcol)
                wg = pool.tile([P, N], f32, tag="wg")
                nc.vector.scalar_tensor_tensor(wg, tt_, T_MIN, tt_, alu.is_gt, alu.mult)
                w = pool.tile([P, N], f32, tag="w")
                nc.vector.tensor_tensor(w, wg, va, alu.mult)
                # outputs: rgb(3), nrm(3), dep, alpha_acc
                yo = it * 8
                for c in range(6):
                    nc.vector.scalar_tensor_tensor(
                        junk, w, 1.0, bc[c + 1], alu.mult, alu.mult,
                        accum_out=y_all[:, yo + c : yo + c + 1],
                    )
                nc.vector.scalar_tensor_tensor(
                    junk, w, 1.0, tmv, alu.mult, alu.mult,
                    accum_out=y_all[:, yo + 6 : yo + 7],
                )
                nc.vector.tensor_reduce(y_all[:, yo + 7 : yo + 8], w, X, alu.add)
            nc.sync.dma_start(out=y[:, :], in_=y_all)
    if split_waits:
        _split_excess_waits(nc)
    return nc


def _split_excess_waits(nc):
    # This walrus build encodes at most ONE semaphore wait per instruction
    # ("Too many sync wait commands"). Two-step fix:
    #  1. drop waits on the instruction's own engine-completion semaphore
    #     (same-engine program order already guarantees them);
    #  2. move any remaining excess waits onto wait-only Drain carriers
    #     injected just before the instruction on the same engine.
    import concourse.mybir as mybir

    own_sem_prefix = {
        mybir.EngineType.PE: "PE",
        mybir.EngineType.Activation: "Activation",
        mybir.EngineType.DVE: "DVE",
        mybir.EngineType.Pool: "Pool",
        mybir.EngineType.SP: "SP",
    }

    cnt = 0
    for f in nc.m.functions:
        for b in f.blocks:
            il = list(b.instructions)
            out = []
            changed = False
            for ins in il:
                si = ins.sync_info
                try:
                    waits = list(si.on_wait)
                except Exception:
                    waits = []
                if len(waits) > 1:
                    pfx = own_sem_prefix.get(ins.engine)
                    if pfx is not None:
                        keep = [
                            wx
                            for wx in waits
                            if not (wx.ant_name or "").startswith(pfx + "_")
                        ]
                        if keep:
                            waits = keep
                    for wx in waits[:-1]:
                        nop = mybir.InstDrain(
                            name=f"Wsplit-{cnt}",
                            ins=[],
                            outs=[],
                            bass_is_fusable=False,
                        )
                        nop.engine = ins.engine
                        nop.sync_info = mybir.SyncInfo(on_wait=[wx], on_update=[])
                        cnt += 1
                        out.append(nop)
                    si.on_wait = waits[-1:]
                    ins.sync_info = si
                    changed = True
                out.append(ins)
            if changed:
                b.instructions = out
    return nc


def _build_runtime():
    global _rt
    import jax
    from jax.experimental.shard_map import shard_map
    from jax.sharding import Mesh, PartitionSpec

    import concourse.mybir as mybir
    from concourse.bass2jax import (
        _bass_exec_p,
        install_neuronx_cc_hook,
        partition_id_tensor,
    )

    install_neuronx_cc_hook()
    nc = _build_bass(RL, NT)

    partition_name = nc.partition_id_tensor.name if nc.partition_id_tensor else None
    in_names, out_names, out_avals, zero_outs = [], [], [], []
    for alloc in nc.m.functions[0].allocations:
        if not isinstance(alloc, mybir.MemoryLocationSet):
            continue
        name = alloc.memorylocations[0].name
        if alloc.kind == "ExternalInput":
            if name != partition_name:
                in_names.append(name)
        elif alloc.kind == "ExternalOutput":
            shape = tuple(alloc.tensor_shape)
            npdt = np.dtype(mybir.dt.np(alloc.dtype))
            out_names.append(name)
            out_avals.append(jax.core.ShapedArray(shape, npdt))
            zero_outs.append(np.zeros(shape, npdt))

    n_params = len(in_names)
    n_outs = len(out_names)
    all_in_names = list(in_names) + list(out_names)
    if partition_name is not None:
        all_in_names.append(partition_name)
    donate = tuple(range(n_params, n_params + n_outs))

    def _body(*args):
        operands = list(args)
        if partition_name is not None:
            operands.append(partition_id_tensor())
        outs = _bass_exec_p.bind(
            *operands,
            out_avals=tuple(out_avals),
            in_names=tuple(all_in_names),
            out_names=tuple(out_names),
            lowering_input_output_aliases=(),
            sim_require_finite=True,
            sim_require_nnan=True,
            nc=nc,
        )
        return tuple(outs)

    devices = jax.devices()[:M]
    mesh = Mesh(np.asarray(devices), ("core",))
    in_specs = (PartitionSpec("core"),) * (n_params + n_outs)
    out_specs = (PartitionSpec("core"),) * n_outs
    sharded = jax.jit(
        shard_map(_body, mesh=mesh, in_specs=in_specs, out_specs=out_specs,
                  check_rep=False),
        donate_argnums=donate,
        keep_unused=True,
    )
    _rt = dict(
        sharded=sharded,
        in_names=in_names,
        out_names=out_names,
        zero_shapes=[(z.shape, z.dtype) for z in zero_outs],
    )
    return _rt


def _host_prep(rgs_xyz, rgs_rot, rgs_sca, rgs_opa, rgs_nrm, ray_org, ray_dir):
    f32 = np.float32
    xyz = np.asarray(rgs_xyz, f32)[0]
    rot = np.asarray(rgs_rot, f32)[0]
    sca = np.asarray(rgs_sca, f32)[0]
    opa = np.asarray(rgs_opa, f32)[0, :, 0]
    nrm = np.asarray(rgs_nrm, f32)[0]
    ro = np.asarray(ray_org, f32).reshape(3)
    rd = np.asarray(ray_dir, f32).reshape(R, 3)

    q = rot / np.sqrt(np.sum(rot * rot, axis=-1, keepdims=True) + 1e-12)
    w_, x_, y_, z_ = q[:, 0], q[:, 1], q[:, 2], q[:, 3]
    Rm = np.stack(
        [
            1 - 2 * (y_ * y_ + z_ * z_), 2 * (x_ * y_ - w_ * z_), 2 * (x_ * z_ + w_ * y_),
            2 * (x_ * y_ + w_ * z_), 1 - 2 * (x_ * x_ + z_ * z_), 2 * (y_ * z_ - w_ * x_),
            2 * (x_ * z_ - w_ * y_), 2 * (y_ * z_ + w_ * x_), 1 - 2 * (x_ * x_ + y_ * y_),
        ],
        axis=-1,
    ).reshape(N, 3, 3).astype(f32)
    Minv = (np.swapaxes(Rm, -1, -2) / sca[:, :, None]).astype(f32)
    o_loc = np.einsum("nij,nj->ni", Minv, ro[None, :] - xyz).astype(f32)
    v = np.einsum("nij,ni->nj", Minv, o_loc).astype(f32)
    Q = np.einsum("nki,nkj->nij", Minv, Minv).astype(f32)
    oo = np.sum(o_loc * o_loc, axis=-1).astype(f32)
    Q6 = np.stack(
        [Q[:, 0, 0], Q[:, 1, 1], Q[:, 2, 2],
         2 * Q[:, 0, 1], 2 * Q[:, 0, 2], 2 * Q[:, 1, 2]],
        axis=0,
    ).astype(f32)

    dx, dy, dz = rd[:, 0], rd[:, 1], rd[:, 2]
    F = np.stack([dx * dx, dy * dy, dz * dz, dx * dy, dx * dz, dy * dz], axis=1)
    F = np.ascontiguousarray(F, f32)

    nrm_u = (nrm / np.sqrt(np.sum(nrm * nrm, axis=-1, keepdims=True) + 1e-12)).astype(f32)
    return rd, F, v.T.copy(), Q6, oo, opa, nrm_u


def kernel(rgs_xyz, rgs_rot, rgs_sca, rgs_opa, rgs_rgb, rgs_nrm, bg_raw, ray_org, ray_dir):
    f32 = np.float32
    rt = _rt if _rt is not None else _build_runtime()

    rd, F, vT, Q6, oo, opa, nrm_u = _host_prep(
        rgs_xyz, rgs_rot, rgs_sca, rgs_opa, rgs_nrm, ray_org, ray_dir
    )
    rgb = np.asarray(rgs_rgb, f32)[0]
    rows = np.empty((16, N), f32)
    rows[0] = np.log(opa) - 0.5 * oo
    rows[1:4] = rgb.T
    rows[4:7] = nrm_u.T
    rows[7:10] = vT
    rows[10:16] = Q6

    rdf = np.concatenate([rd, F], axis=1)  # [R, 9]
    # per core: [RL, 9] -> [NT, P, 9] -> [P, NT*9]
    rdf_packed = [
        np.ascontiguousarray(
            rdf[c * RL : (c + 1) * RL].reshape(NT, P, 9).transpose(1, 0, 2).reshape(P, NT * 9)
        )
        for c in range(M)
    ]
    per_core = {
        "rdf": rdf_packed,
        "rows": [rows] * M,
    }
    concat_in = [
        np.ascontiguousarray(np.concatenate(per_core[name], axis=0))
        for name in rt["in_names"]
    ]
    zeros = [np.zeros((M * s[0], *s[1:]), d) for s, d in rt["zero_shapes"]]
    out_arrs = rt["sharded"](*concat_in, *zeros)
    yk = rt["out_names"].index("y")
    # [M*P, NT*8] -> per core [P, NT, 8] -> [NT, P, 8] -> rays in order
    yraw = np.asarray(out_arrs[yk]).reshape(M, P, NT, 8)
    yall = yraw.transpose(0, 2, 1, 3).reshape(R, 8)

    img = yall[:, 0:3]
    nrm_acc = yall[:, 3:6]
    dep = yall[:, 6:7]
    aacc = yall[:, 7:8]
    bg = np.broadcast_to(np.asarray(bg_raw, f32), (B, H, W, 3)).reshape(R, 3)
    image = img + (1.0 - aacc) * bg
    normal = nrm_acc / np.sqrt(np.sum(nrm_acc * nrm_acc, axis=-1, keepdims=True) + 1e-12)
    out = np.concatenate([image, aacc, dep, normal], axis=-1).astype(f32)
    return out.reshape(B, H, W, 8)
